# revision 13
# baseline (speedup 1.0000x reference)
"""Multi-head attention (B=4, S=2048, D=1024, H=16) on 8 TRN2 NeuronCores.

Sharding: core c handles batch b = c//2 and head-group hg = c%2 (8 heads).
Tensor-parallel within the core pair of a batch: w_q/w_k/w_v column-split,
w_o row-split; host sums the two partial out-projections per batch.

Device algorithm (per core), feature-major ("transposed") layouts:
  QhT/KhT = W @ x^T in [dims, S] slabs; scores^T[k, q] per head-pair via
  row-tiled concurrent matmuls (K=64 at PE rows 0-63 / 64-127); exp on ACT
  (softmax has no max-subtraction: scores ~N(0,1)); attn@V as col-tiled
  concurrent matmuls (M=64 at PE cols 0-63 / 64-127) accumulating over key
  tiles in one PSUM bank; softmax denominators via DVE tile pre-sum + a
  replicated-ones matmul that lands the denominator pre-broadcast in PSUM;
  normalize on DVE (fast reciprocal + multiply) straight out of PSUM.
  Q/K biases are applied with K=1 rank-1 matmuls inside the projection
  accumulation; the V bias is folded into the host-side output bias
  (w_o @ b_v + b_o). Projection and out-projection matmuls are interleaved
  into the attention phase as PE fillers so the scalar engine (exp) stays
  the pacing engine.
"""

import numpy as np
import ml_dtypes
from collections import deque
from contextlib import ExitStack

import concourse.bass as bass
import concourse.tile as tile
from concourse import bacc, mybir
from concourse.bass_utils import run_bass_kernel_spmd

BF16 = ml_dtypes.bfloat16
F32 = np.float32

D = 1024
N_HEAD = 16
DH = 64
HPC = 8          # heads per core
HW = HPC * DH    # head-group width = 512
P = 128

TRACE = False    # set by test.py for profiling runs

_PROG = {}


def _build_program(S):
    dt = mybir.dt
    bf = dt.bfloat16
    f32 = dt.float32

    CH = 512                 # q-chunk width
    NCH = S // CH            # q-chunks (4)
    NT = S // P              # k-tiles (16)
    NI = D // P              # contraction tiles over model dim (8)
    NP = HPC // 2            # head pairs (4)
    NO = D // P              # out-proj o-tiles (8)

    nc = bacc.Bacc("TRN2", target_bir_lowering=False, debug=False)

    xq = nc.dram_tensor("xq", [D, S], bf, kind="ExternalInput")
    xk = nc.dram_tensor("xk", [D, S], bf, kind="ExternalInput")
    xv = nc.dram_tensor("xv", [D, S], bf, kind="ExternalInput")
    wq = nc.dram_tensor("wq", [D, HW], bf, kind="ExternalInput")
    wk = nc.dram_tensor("wk", [D, HW], bf, kind="ExternalInput")
    wv = nc.dram_tensor("wv", [D, HW], bf, kind="ExternalInput")
    bq = nc.dram_tensor("bq", [1, HW], bf, kind="ExternalInput")
    bk = nc.dram_tensor("bk", [1, HW], bf, kind="ExternalInput")
    wo = nc.dram_tensor("wo", [HW, D], bf, kind="ExternalInput")
    yT = nc.dram_tensor("yT", [D, S], bf, kind="ExternalOutput")

    AF = mybir.ActivationFunctionType

    with tile.TileContext(nc) as tc:
        with ExitStack() as ctx:
            consts = ctx.enter_context(tc.tile_pool(name="consts", bufs=1))
            wpool = ctx.enter_context(tc.tile_pool(name="wpool", bufs=1))
            xpool = ctx.enter_context(tc.tile_pool(name="xpool", bufs=1))
            xqpool = ctx.enter_context(tc.tile_pool(name="xqpool", bufs=16))
            slabs = ctx.enter_context(tc.tile_pool(name="slabs", bufs=1))
            epool = ctx.enter_context(tc.tile_pool(name="epool", bufs=6))
            dpool = ctx.enter_context(tc.tile_pool(name="dpool", bufs=4))
            rpool = ctx.enter_context(tc.tile_pool(name="rpool", bufs=2))
            ypool = ctx.enter_context(tc.tile_pool(name="ypool", bufs=2))
            ps_sc = ctx.enter_context(
                tc.tile_pool(name="ps_sc", bufs=2, space="PSUM"))
            ps_acc = ctx.enter_context(
                tc.tile_pool(name="ps_acc", bufs=2, space="PSUM"))
            ps_proj = ctx.enter_context(
                tc.tile_pool(name="ps_proj", bufs=2, space="PSUM"))

            # ---- constants ----
            ones_row = consts.tile([1, CH], bf)
            nc.vector.memset(ones_row[:], 1.0)
            ones64 = consts.tile([P, DH], bf)
            nc.vector.memset(ones64[:], 1.0)
            warm = consts.tile([1, 16], f32)
            nc.vector.memset(warm[:], 0.0)
            bq_sb = consts.tile([1, HW], bf)
            bk_sb = consts.tile([1, HW], bf)

            # ---- weights / activations (SBUF) ----
            wq_sb = wpool.tile([P, NI, HW], bf)
            wk_sb = wpool.tile([P, NI, HW], bf)
            wv_sb = wpool.tile([P, NI, HW], bf)
            wo_sb = wpool.tile([P, NP, D], bf)
            xk_sb = xpool.tile([P, NI, S], bf)
            xv_sb = xpool.tile([P, NI, S], bf)

            q_slab = slabs.tile([P, NP, S], bf)
            k_slab = slabs.tile([P, NP, S], bf)
            v_sb = slabs.tile([P, NT, HW], bf)
            attn_sb = slabs.tile([P, NP, S], bf)

            # ---- DMA schedule (sync queue; order = priority) ----
            # xv/wv first: V-proj tasks head the PE FIFO, so their data must
            # land before the q/k data that later instructions consume.
            nc.sync.dma_start(bk_sb[:], bk.ap())
            nc.sync.dma_start(bq_sb[:], bq.ap())
            for i in range(NI):   # xv half 0 (tokens 0-1023)
                nc.sync.dma_start(xv_sb[:, i, 0:1024],
                                  xv.ap()[i * P:(i + 1) * P, 0:1024])
            for i in range(NI):
                nc.sync.dma_start(wv_sb[:, i, :], wv.ap()[i * P:(i + 1) * P, :])
            for i in range(NI):
                nc.sync.dma_start(wk_sb[:, i, :], wk.ap()[i * P:(i + 1) * P, :])
            for i in range(NI):   # xk chunk 0 (keys 0-511)
                nc.sync.dma_start(xk_sb[:, i, 0:CH],
                                  xk.ap()[i * P:(i + 1) * P, 0:CH])
            for i in range(NI):
                nc.sync.dma_start(wq_sb[:, i, :], wq.ap()[i * P:(i + 1) * P, :])
            xq_t = {}
            for i in range(NI):   # xq chunk 0
                xq_t[(i, 0)] = xqpool.tile([P, CH], bf, tag="xq",
                                           name=f"xq{i}c0")
                nc.sync.dma_start(xq_t[(i, 0)][:],
                                  xq.ap()[i * P:(i + 1) * P, 0:CH])
            for i in range(NI):   # xv half 1
                nc.sync.dma_start(xv_sb[:, i, 1024:2048],
                                  xv.ap()[i * P:(i + 1) * P, 1024:2048])
            for tcc in range(1, NCH):   # xk chunks 1-3
                for i in range(NI):
                    nc.sync.dma_start(
                        xk_sb[:, i, tcc * CH:(tcc + 1) * CH],
                        xk.ap()[i * P:(i + 1) * P, tcc * CH:(tcc + 1) * CH])
            for c in range(NP):
                nc.sync.dma_start(wo_sb[:, c, :], wo.ap()[c * P:(c + 1) * P, :])
            # xq chunks 1-3 are pool-gated (WAR waits) -> keep them off the
            # sync queue so they can't block later output DMAs.
            for c in range(1, NCH):
                for i in range(NI):
                    xq_t[(i, c)] = xqpool.tile([P, CH], bf, tag="xq",
                                               name=f"xq{i}c{c}")
                    nc.gpsimd.dma_start(xq_t[(i, c)][:],
                                        xq.ap()[i * P:(i + 1) * P,
                                                c * CH:(c + 1) * CH])

            # preload the exp table set early
            nc.scalar.activation(warm[:], warm[:], AF.Exp)

            # ---- filler tasks ----
            def k_task(p, tcc):
                tsl = slice(tcc * CH, (tcc + 1) * CH)

                def go():
                    ps = ps_proj.tile([P, CH], f32, tag="proj")
                    for i in range(NI):
                        nc.tensor.matmul(
                            ps[:], lhsT=wk_sb[:, i, p * P:(p + 1) * P],
                            rhs=xk_sb[:, i, tsl], start=(i == 0), stop=False)
                    nc.tensor.matmul(
                        ps[:], lhsT=bk_sb[0:1, p * P:(p + 1) * P],
                        rhs=ones_row[:], start=False, stop=True)
                    nc.vector.tensor_copy(k_slab[:, p, tsl], ps[:])
                return go

            def q_task(p, c):
                csl = slice(c * CH, (c + 1) * CH)

                def go():
                    ps = ps_proj.tile([P, CH], f32, tag="proj")
                    for i in range(NI):
                        nc.tensor.matmul(
                            ps[:], lhsT=wq_sb[:, i, p * P:(p + 1) * P],
                            rhs=xq_t[(i, c)][:], start=(i == 0), stop=False)
                    nc.tensor.matmul(
                        ps[:], lhsT=bq_sb[0:1, p * P:(p + 1) * P],
                        rhs=ones_row[:], start=False, stop=True)
                    nc.vector.tensor_copy(q_slab[:, p, csl], ps[:])
                return go

            def v_task(t):
                tsl = slice(t * P, (t + 1) * P)

                def go():
                    ps = ps_proj.tile([P, HW], f32, tag="proj")
                    for i in range(NI):
                        nc.tensor.matmul(
                            ps[:], lhsT=xv_sb[:, i, tsl], rhs=wv_sb[:, i, :],
                            start=(i == 0), stop=(i == NI - 1))
                    nc.vector.tensor_copy(v_sb[:, t, :], ps[:])
                return go

            def o_task(o, c):
                csl = slice(c * CH, (c + 1) * CH)

                def go():
                    ps = ps_proj.tile([P, CH], f32, tag="proj")
                    for ci in range(NP):
                        nc.tensor.matmul(
                            ps[:], lhsT=wo_sb[:, ci, o * P:(o + 1) * P],
                            rhs=attn_sb[:, ci, csl],
                            start=(ci == 0), stop=(ci == NP - 1))
                    st = ypool.tile([P, CH], bf, tag="yst")
                    nc.vector.tensor_copy(st[:], ps[:])
                    nc.sync.dma_start(yT.ap()[o * P:(o + 1) * P, csl], st[:])
                return go

            fillers = deque()

            def pump(n=1):
                for _ in range(n):
                    if fillers:
                        fillers.popleft()()

            # ---- attention iteration ----
            # The softmax epilogue (denominator matmuls -> reciprocal ->
            # normalize) of iteration N is deferred into iteration N+1's
            # kt==2 slot: its denominator matmuls wait on the DVE pre-sum
            # chain, and emitting them at the iteration boundary would
            # head-of-line-block the PE queue for ~2.8us per iteration.
            pending_epi = [None]

            def attention(p, c, kt_hook=None):
                csl = slice(c * CH, (c + 1) * CH)
                hA, hB = 2 * p, 2 * p + 1
                acc = ps_acc.tile([P, CH], f32, tag="acc")
                dprev = None
                pend = deque()

                def issue_av(et, kt):
                    nc.tensor.matmul(
                        acc[0:64, :], lhsT=v_sb[:, kt, hA * DH:(hA + 1) * DH],
                        rhs=et[:, 0:CH], start=(kt == 0), stop=(kt == NT - 1),
                        tile_position=(0, 0))
                    nc.tensor.matmul(
                        acc[64:128, :], lhsT=v_sb[:, kt, hB * DH:(hB + 1) * DH],
                        rhs=et[:, CH:2 * CH], start=(kt == 0),
                        stop=(kt == NT - 1), tile_position=(0, 64))

                for kt in range(NT):
                    if kt == 2 and pending_epi[0] is not None:
                        pending_epi[0]()
                        pending_epi[0] = None
                    if kt_hook is not None:
                        kt_hook(kt)
                    ksl = slice(kt * P, (kt + 1) * P)
                    ps = ps_sc.tile([P, 2 * CH], f32, tag="sc")
                    nc.tensor.matmul(
                        ps[:, 0:CH], lhsT=k_slab[0:64, p, ksl],
                        rhs=q_slab[0:64, p, csl],
                        start=True, stop=True, tile_position=(0, 0))
                    nc.tensor.matmul(
                        ps[:, CH:2 * CH], lhsT=k_slab[64:128, p, ksl],
                        rhs=q_slab[64:128, p, csl],
                        start=True, stop=True, tile_position=(64, 0))
                    et = epool.tile([P, 2 * CH], bf, tag="et")
                    nc.scalar.activation(et[:], ps[:], AF.Exp)
                    pend.append((et, kt))
                    if len(pend) == 3:
                        issue_av(*pend.popleft())
                    dcur = dpool.tile([P, 2 * CH], bf, tag="dsum")
                    if kt == 0:
                        nc.vector.tensor_copy(dcur[:], et[:])
                    else:
                        nc.vector.tensor_add(dcur[:], dprev[:], et[:])
                    dprev = dcur
                    if kt_hook is None:
                        pump()
                while pend:
                    issue_av(*pend.popleft())

                dsum = dprev

                def epilogue():
                    # denominators, pre-broadcast: rows 0-63 <- A, 64-127 <- B
                    dps = ps_sc.tile([P, 2 * CH], f32, tag="sc")
                    nc.tensor.matmul(dps[0:64, 0:CH], lhsT=ones64[:],
                                     rhs=dsum[:, 0:CH], start=True, stop=True,
                                     tile_position=(0, 0))
                    nc.tensor.matmul(dps[64:128, 0:CH], lhsT=ones64[:],
                                     rhs=dsum[:, CH:2 * CH], start=True,
                                     stop=True, tile_position=(0, 64))
                    rcp = rpool.tile([P, CH], f32, tag="rcp")
                    nc.vector.reciprocal_approx_fast(rcp[:], dps[:, 0:CH])
                    nc.vector.tensor_mul(attn_sb[:, p, csl], acc[:], rcp[:])

                pending_epi[0] = epilogue

            # ---- prefix: first V tiles, K(p0) chunk 0, Q(p0,c0) ----
            v_task(0)()
            v_task(1)()
            v_task(2)()
            k_task(0, 0)()
            q_task(0, 0)()

            # ---- iteration 0: (p0, c0) with V/K just-in-time ----
            k_remaining0 = deque([k_task(0, tcc) for tcc in (1, 2, 3)])

            def hook0(kt):
                # V(t) three steps ahead of its AV; K(p0) chunks ahead of use
                if kt < NT - 3:
                    v_task(kt + 3)()
                if kt in (1, 5, 9) and k_remaining0:
                    k_remaining0.popleft()()
                if kt % 3 == 0:
                    pump()

            # filler queue: ordered by deadline
            for tcc in range(NCH):
                fillers.append(k_task(1, tcc))
            fillers.append(q_task(1, 0))
            for tcc in range(NCH):
                fillers.append(k_task(2, tcc))
            fillers.append(q_task(2, 0))
            for tcc in range(NCH):
                fillers.append(k_task(3, tcc))
            fillers.append(q_task(3, 0))

            attention(0, 0, kt_hook=hook0)

            # remaining iterations, c-outer / p-inner
            for c in range(NCH):
                for p in range(NP):
                    if c + 1 < NCH:
                        fillers.append(q_task(p, c + 1))
                    if p == NP - 1 and c > 0:
                        for o in range(NO):
                            fillers.append(o_task(o, c - 1))
                    if c == 0 and p == 0:
                        continue  # ran above with the V/K hook
                    attention(p, c)

            # tail: last epilogue, remaining fillers, final out-proj
            if pending_epi[0] is not None:
                pending_epi[0]()
                pending_epi[0] = None
            while fillers:
                pump()
            for o in range(NO):
                o_task(o, NCH - 1)()

    nc.compile()
    return nc


def _get_program(S):
    if S not in _PROG:
        _PROG[S] = _build_program(S)
    return _PROG[S]


def enable_trace():
    """Register the NTFF profiling hook (axon images lack antenv.axon_hooks)
    and neuter the cloud artifact upload; then TRACE=True runs return
    exec_time_ns."""
    global TRACE
    import sys
    import types
    import antenv
    if "antenv.axon_hooks" not in sys.modules:
        _m = types.ModuleType("antenv.axon_hooks")
        _m._hook = None
        _m.set_axon_ntff_profile_hook = lambda h: setattr(_m, "_hook", h)
        _m.get_axon_ntff_profile_hook = lambda: _m._hook
        sys.modules["antenv.axon_hooks"] = _m
        antenv.axon_hooks = _m
        from trn_agent_boot.trn_boot import _ntff_profile_via_ctypes
        _m._hook = _ntff_profile_via_ctypes("/opt/axon/libaxon_pjrt.so")
    import concourse.bass_utils as bu
    bu.upload_artifacts = lambda tmpdir: tmpdir
    TRACE = True


def kernel(q, k, v, w_q, b_q, w_k, b_k, w_v, b_v, w_o, b_o):
    q, k, v = (np.asarray(a, F32) for a in (q, k, v))
    w_q, b_q, w_k, b_k = (np.asarray(a, F32) for a in (w_q, b_q, w_k, b_k))
    w_v, b_v, w_o, b_o = (np.asarray(a, F32) for a in (w_v, b_v, w_o, b_o))
    B, S, _ = q.shape

    nc = _get_program(S)

    scale = 1.0 / np.sqrt(np.float32(DH))
    n_cores = 2 * B
    in_maps = []
    for core in range(n_cores):
        b, hg = core // 2, core % 2
        hsl = slice(hg * HW, (hg + 1) * HW)
        m = {
            "xq": np.ascontiguousarray(q[b].T).astype(BF16),
            "xk": np.ascontiguousarray(k[b].T).astype(BF16),
            "xv": np.ascontiguousarray(v[b].T).astype(BF16),
            "wq": np.ascontiguousarray((w_q[hsl] * scale).T).astype(BF16),
            "wk": np.ascontiguousarray(w_k[hsl].T).astype(BF16),
            "wv": np.ascontiguousarray(w_v[hsl].T).astype(BF16),
            "bq": np.ascontiguousarray((b_q[hsl] * scale)[None, :]).astype(BF16),
            "bk": np.ascontiguousarray(b_k[hsl][None, :]).astype(BF16),
            "wo": np.ascontiguousarray(w_o[:, hsl].T).astype(BF16),
        }
        in_maps.append(m)

    res = run_bass_kernel_spmd(nc, in_maps, list(range(n_cores)), trace=TRACE)

    bias = b_o + w_o @ b_v
    out = np.empty((B, S, D), F32)
    for b in range(B):
        yt = (res.results[2 * b]["yT"].astype(F32)
              + res.results[2 * b + 1]["yT"].astype(F32))
        out[b] = yt.T + bias
    if TRACE:
        kernel.last_exec_time_ns = res.exec_time_ns
    return out


# revision 21
# speedup vs baseline: 1.0060x; 1.0060x over previous
"""Multi-head attention (B=4, S=2048, D=1024, H=16) on 8 TRN2 NeuronCores.

Sharding: core c handles batch b = c//2 and head-group hg = c%2 (8 heads).
Tensor-parallel within the core pair of a batch: w_q/w_k/w_v column-split,
w_o row-split; host sums the two partial out-projections per batch.

Device algorithm (per core), feature-major ("transposed") layouts:
  QhT/KhT = W @ x^T in [dims, S] slabs; scores^T[k, q] per head-pair via
  row-tiled concurrent matmuls (K=64 at PE rows 0-63 / 64-127); exp on ACT
  (softmax has no max-subtraction: scores ~N(0,1)); attn@V as col-tiled
  concurrent matmuls (M=64 at PE cols 0-63 / 64-127) accumulating over key
  tiles in one PSUM bank; softmax denominators via DVE tile pre-sum + a
  replicated-ones matmul that lands the denominator pre-broadcast in PSUM;
  normalize on DVE (fast reciprocal + multiply) straight out of PSUM.
  Q/K biases are applied with K=1 rank-1 matmuls inside the projection
  accumulation; the V bias is folded into the host-side output bias
  (w_o @ b_v + b_o). Projection and out-projection matmuls are interleaved
  into the attention phase as PE fillers so the scalar engine (exp) stays
  the pacing engine.
"""

import numpy as np
import ml_dtypes
from collections import deque
from contextlib import ExitStack

import concourse.bass as bass
import concourse.tile as tile
from concourse import bacc, mybir
from concourse.bass_utils import run_bass_kernel_spmd

BF16 = ml_dtypes.bfloat16
F32 = np.float32

D = 1024
N_HEAD = 16
DH = 64
HPC = 8          # heads per core
HW = HPC * DH    # head-group width = 512
P = 128

TRACE = False    # set by test.py for profiling runs

_PROG = {}


def _build_program(S):
    dt = mybir.dt
    bf = dt.bfloat16
    f32 = dt.float32

    CH = 512                 # q-chunk width
    NCH = S // CH            # q-chunks (4)
    NT = S // P              # k-tiles (16)
    NI = D // P              # contraction tiles over model dim (8)
    NP = HPC // 2            # head pairs (4)
    NO = D // P              # out-proj o-tiles (8)

    nc = bacc.Bacc("TRN2", target_bir_lowering=False, debug=False)

    xq = nc.dram_tensor("xq", [D, S], bf, kind="ExternalInput")
    xk = nc.dram_tensor("xk", [D, S], bf, kind="ExternalInput")
    xv = nc.dram_tensor("xv", [D, S], bf, kind="ExternalInput")
    wq = nc.dram_tensor("wq", [D, HW], bf, kind="ExternalInput")
    wk = nc.dram_tensor("wk", [D, HW], bf, kind="ExternalInput")
    wv = nc.dram_tensor("wv", [D, HW], bf, kind="ExternalInput")
    bq = nc.dram_tensor("bq", [1, HW], bf, kind="ExternalInput")
    bk = nc.dram_tensor("bk", [1, HW], bf, kind="ExternalInput")
    wo = nc.dram_tensor("wo", [HW, D], bf, kind="ExternalInput")
    yT = nc.dram_tensor("yT", [D, S], bf, kind="ExternalOutput")

    AF = mybir.ActivationFunctionType

    with tile.TileContext(nc) as tc:
        with ExitStack() as ctx:
            consts = ctx.enter_context(tc.tile_pool(name="consts", bufs=1))
            wpool = ctx.enter_context(tc.tile_pool(name="wpool", bufs=1))
            xpool = ctx.enter_context(tc.tile_pool(name="xpool", bufs=1))
            slabs = ctx.enter_context(tc.tile_pool(name="slabs", bufs=1))
            epool = ctx.enter_context(tc.tile_pool(name="epool", bufs=6))
            dpool = ctx.enter_context(tc.tile_pool(name="dpool", bufs=3))
            rpool = ctx.enter_context(tc.tile_pool(name="rpool", bufs=2))
            ypool = ctx.enter_context(tc.tile_pool(name="ypool", bufs=2))
            ps_sc = ctx.enter_context(
                tc.tile_pool(name="ps_sc", bufs=2, space="PSUM"))
            ps_acc = ctx.enter_context(
                tc.tile_pool(name="ps_acc", bufs=2, space="PSUM"))
            ps_proj = ctx.enter_context(
                tc.tile_pool(name="ps_proj", bufs=2, space="PSUM"))

            # ---- constants ----
            ones_row = consts.tile([1, CH], bf)
            nc.vector.memset(ones_row[:], 1.0)
            ones64 = consts.tile([P, DH], bf)
            nc.vector.memset(ones64[:], 1.0)
            warm = consts.tile([1, 16], f32)
            nc.vector.memset(warm[:], 0.0)
            bq_sb = consts.tile([1, HW], bf)
            bk_sb = consts.tile([1, HW], bf)

            # ---- weights / activations (SBUF) ----
            wq_sb = wpool.tile([P, NI, HW], bf)
            wk_sb = wpool.tile([P, NI, HW], bf)
            wv_sb = wpool.tile([P, NI, HW], bf)
            wo_sb = wpool.tile([P, NP, D], bf)
            xk_sb = xpool.tile([P, NI, S], bf)
            xq_sb = xpool.tile([P, NI, S], bf)
            xv_sb = xpool.tile([P, NI, 1024], bf)   # half-resident, 2 waves

            q_slab = slabs.tile([P, NP, S], bf)
            k_slab = slabs.tile([P, NP, S], bf)
            v_sb = slabs.tile([P, NT, HW], bf)
            attn_sb = slabs.tile([P, NP, S], bf)

            # ---- DMA schedule ----
            # Two queues issue in parallel (~0.7us per dma_start each);
            # full-row [128, 2048] transfers hit all 16 SBUF ports.
            # sync:   wk, xq, xk, wo  (+ y outputs appended later)
            # gpsimd: wq, wv, xv wave 0, xv wave 1 (gated on V(t7))
            nc.sync.dma_start(bk_sb[:], bk.ap())
            nc.sync.dma_start(bq_sb[:], bq.ap())
            for i in range(NI):
                nc.sync.dma_start(wk_sb[:, i, :], wk.ap()[i * P:(i + 1) * P, :])
            for i in range(NI):
                nc.sync.dma_start(xq_sb[:, i, :], xq.ap()[i * P:(i + 1) * P, :])
            for i in range(NI):
                nc.sync.dma_start(xk_sb[:, i, :], xk.ap()[i * P:(i + 1) * P, :])
            for c in range(NP):
                nc.sync.dma_start(wo_sb[:, c, :], wo.ap()[c * P:(c + 1) * P, :])
            for i in range(NI):
                nc.gpsimd.dma_start(wq_sb[:, i, :],
                                    wq.ap()[i * P:(i + 1) * P, :])
            for i in range(NI):
                nc.gpsimd.dma_start(wv_sb[:, i, :],
                                    wv.ap()[i * P:(i + 1) * P, :])
            for i in range(NI):   # xv wave 0 (tokens 0-1023)
                nc.gpsimd.dma_start(xv_sb[:, i, :],
                                    xv.ap()[i * P:(i + 1) * P, 0:1024])

            def xv_wave1():
                for i in range(NI):   # overwrites wave 0 (gated on V(t7))
                    nc.gpsimd.dma_start(xv_sb[:, i, :],
                                        xv.ap()[i * P:(i + 1) * P, 1024:2048])

            # preload the exp table set early
            nc.scalar.activation(warm[:], warm[:], AF.Exp)

            # ---- filler tasks ----
            def k_task(p, tcc):
                tsl = slice(tcc * CH, (tcc + 1) * CH)

                def go():
                    ps = ps_proj.tile([P, CH], f32, tag="proj")
                    for i in range(NI):
                        nc.tensor.matmul(
                            ps[:], lhsT=wk_sb[:, i, p * P:(p + 1) * P],
                            rhs=xk_sb[:, i, tsl], start=(i == 0), stop=False)
                    nc.tensor.matmul(
                        ps[:], lhsT=bk_sb[0:1, p * P:(p + 1) * P],
                        rhs=ones_row[:], start=False, stop=True)
                    nc.vector.tensor_copy(k_slab[:, p, tsl], ps[:])
                return go

            def q_task(p, c):
                csl = slice(c * CH, (c + 1) * CH)

                def go():
                    ps = ps_proj.tile([P, CH], f32, tag="proj")
                    for i in range(NI):
                        nc.tensor.matmul(
                            ps[:], lhsT=wq_sb[:, i, p * P:(p + 1) * P],
                            rhs=xq_sb[:, i, csl], start=(i == 0), stop=False)
                    nc.tensor.matmul(
                        ps[:], lhsT=bq_sb[0:1, p * P:(p + 1) * P],
                        rhs=ones_row[:], start=False, stop=True)
                    nc.vector.tensor_copy(q_slab[:, p, csl], ps[:])
                return go

            def v_task(t):
                tsl = slice((t % 8) * P, (t % 8 + 1) * P)

                def go():
                    ps = ps_proj.tile([P, HW], f32, tag="proj")
                    for i in range(NI):
                        nc.tensor.matmul(
                            ps[:], lhsT=xv_sb[:, i, tsl], rhs=wv_sb[:, i, :],
                            start=(i == 0), stop=(i == NI - 1))
                    nc.vector.tensor_copy(v_sb[:, t, :], ps[:])
                return go

            def o_task(o, c):
                csl = slice(c * CH, (c + 1) * CH)

                def go():
                    ps = ps_proj.tile([P, CH], f32, tag="proj")
                    for ci in range(NP):
                        nc.tensor.matmul(
                            ps[:], lhsT=wo_sb[:, ci, o * P:(o + 1) * P],
                            rhs=attn_sb[:, ci, csl],
                            start=(ci == 0), stop=(ci == NP - 1))
                    st = ypool.tile([P, CH], bf, tag="yst")
                    nc.vector.tensor_copy(st[:], ps[:])
                    nc.sync.dma_start(yT.ap()[o * P:(o + 1) * P, csl], st[:])
                return go

            fillers = deque()

            def pump(n=1):
                for _ in range(n):
                    if fillers:
                        fillers.popleft()()

            # ---- attention iteration ----
            # The softmax epilogue (denominator matmuls -> reciprocal ->
            # normalize) of iteration N is deferred into iteration N+1's
            # kt==2 slot: its denominator matmuls wait on the DVE pre-sum
            # chain, and emitting them at the iteration boundary would
            # head-of-line-block the PE queue for ~2.8us per iteration.
            pending_epi = [None]

            def attention(p, c, kt_hook=None, pump_every=3):
                csl = slice(c * CH, (c + 1) * CH)
                hA, hB = 2 * p, 2 * p + 1
                acc = ps_acc.tile([P, CH], f32, tag="acc")
                dprev = None
                pend = deque()

                def issue_av(et, kt):
                    nc.tensor.matmul(
                        acc[0:64, :], lhsT=v_sb[:, kt, hA * DH:(hA + 1) * DH],
                        rhs=et[:, 0:CH], start=(kt == 0), stop=(kt == NT - 1),
                        tile_position=(0, 0))
                    nc.tensor.matmul(
                        acc[64:128, :], lhsT=v_sb[:, kt, hB * DH:(hB + 1) * DH],
                        rhs=et[:, CH:2 * CH], start=(kt == 0),
                        stop=(kt == NT - 1), tile_position=(0, 64))

                for kt in range(NT):
                    if kt == 2 and pending_epi[0] is not None:
                        pending_epi[0]()
                        pending_epi[0] = None
                    if kt_hook is not None:
                        kt_hook(kt)
                    ksl = slice(kt * P, (kt + 1) * P)
                    ps = ps_sc.tile([P, 2 * CH], f32, tag="sc")
                    nc.tensor.matmul(
                        ps[:, 0:CH], lhsT=k_slab[0:64, p, ksl],
                        rhs=q_slab[0:64, p, csl],
                        start=True, stop=True, tile_position=(0, 0))
                    nc.tensor.matmul(
                        ps[:, CH:2 * CH], lhsT=k_slab[64:128, p, ksl],
                        rhs=q_slab[64:128, p, csl],
                        start=True, stop=True, tile_position=(64, 0))
                    et = epool.tile([P, 2 * CH], bf, tag="et")
                    nc.scalar.activation(et[:], ps[:], AF.Exp)
                    pend.append((et, kt))
                    if len(pend) == 3:
                        issue_av(*pend.popleft())
                    dcur = dpool.tile([P, 2 * CH], bf, tag="dsum")
                    if kt == 0:
                        nc.vector.tensor_copy(dcur[:], et[:])
                    else:
                        nc.vector.tensor_add(dcur[:], dprev[:], et[:])
                    dprev = dcur
                    if kt_hook is None and kt % pump_every == 0:
                        pump()
                while pend:
                    issue_av(*pend.popleft())

                dsum = dprev

                def epilogue():
                    # denominators, pre-broadcast: rows 0-63 <- A, 64-127 <- B
                    dps = ps_sc.tile([P, 2 * CH], f32, tag="sc")
                    nc.tensor.matmul(dps[0:64, 0:CH], lhsT=ones64[:],
                                     rhs=dsum[:, 0:CH], start=True, stop=True,
                                     tile_position=(0, 0))
                    nc.tensor.matmul(dps[64:128, 0:CH], lhsT=ones64[:],
                                     rhs=dsum[:, CH:2 * CH], start=True,
                                     stop=True, tile_position=(0, 64))
                    rcp = rpool.tile([P, CH], f32, tag="rcp")
                    nc.vector.reciprocal_approx_fast(rcp[:], dps[:, 0:CH])
                    nc.vector.tensor_mul(attn_sb[:, p, csl], acc[:], rcp[:])

                pending_epi[0] = epilogue

            # ---- prefix: first V tiles, K(p0) chunk 0, Q(p0,c0) ----
            v_task(0)()
            v_task(1)()
            v_task(2)()
            k_task(0, 0)()
            q_task(0, 0)()

            # ---- iteration 0: (p0, c0) with V/K just-in-time ----
            k_remaining0 = deque([k_task(0, tcc) for tcc in (1, 2, 3)])

            def hook0(kt):
                # V(t) three steps ahead of its AV; K(p0) chunks ahead of use
                if kt < NT - 3:
                    v_task(kt + 3)()
                    if kt + 3 == 7:
                        xv_wave1()
                if kt in (2, 6, 10) and k_remaining0:
                    k_remaining0.popleft()()
                if kt % 3 == 0:
                    pump()

            # Q(0,1) must drain inside iteration (0,0): with p-outer order
            # (p,c+1) directly follows (p,c), so each iteration pumps the
            # NEXT chunk's Q projection of its own pair.
            fillers.append(q_task(0, 1))
            attention(0, 0, kt_hook=hook0)

            # remaining iterations, p-outer / c-inner: a new pair's K
            # projection is only due 4 iterations after it is queued, so
            # early iterations stay exp-paced instead of projection-bound.
            for p in range(NP):
                for c in range(NCH):
                    if p == 0 and c == 0:
                        continue  # ran above with the V/K hook
                    if c + 1 < NCH:
                        fillers.append(q_task(p, c + 1))
                    if c == 1 and p + 1 < NP:
                        for tcc in range(NCH):
                            fillers.append(k_task(p + 1, tcc))
                        fillers.append(q_task(p + 1, 0))
                    if p == NP - 1 and c > 0:
                        for o in range(NO):
                            fillers.append(o_task(o, c - 1))
                    attention(p, c, pump_every=(2 if p == NP - 1 else 3))

            # tail: last epilogue, remaining fillers, final out-proj
            if pending_epi[0] is not None:
                pending_epi[0]()
                pending_epi[0] = None
            while fillers:
                pump()
            for o in range(NO):
                o_task(o, NCH - 1)()

    nc.compile()
    return nc


def _get_program(S):
    if S not in _PROG:
        _PROG[S] = _build_program(S)
    return _PROG[S]


def enable_trace():
    """Register the NTFF profiling hook (axon images lack antenv.axon_hooks)
    and neuter the cloud artifact upload; then TRACE=True runs return
    exec_time_ns."""
    global TRACE
    import sys
    import types
    import antenv
    if "antenv.axon_hooks" not in sys.modules:
        _m = types.ModuleType("antenv.axon_hooks")
        _m._hook = None
        _m.set_axon_ntff_profile_hook = lambda h: setattr(_m, "_hook", h)
        _m.get_axon_ntff_profile_hook = lambda: _m._hook
        sys.modules["antenv.axon_hooks"] = _m
        antenv.axon_hooks = _m
        from trn_agent_boot.trn_boot import _ntff_profile_via_ctypes
        _m._hook = _ntff_profile_via_ctypes("/opt/axon/libaxon_pjrt.so")
    import concourse.bass_utils as bu
    bu.upload_artifacts = lambda tmpdir: tmpdir
    TRACE = True


def kernel(q, k, v, w_q, b_q, w_k, b_k, w_v, b_v, w_o, b_o):
    q, k, v = (np.asarray(a, F32) for a in (q, k, v))
    w_q, b_q, w_k, b_k = (np.asarray(a, F32) for a in (w_q, b_q, w_k, b_k))
    w_v, b_v, w_o, b_o = (np.asarray(a, F32) for a in (w_v, b_v, w_o, b_o))
    B, S, _ = q.shape

    nc = _get_program(S)

    scale = 1.0 / np.sqrt(np.float32(DH))
    n_cores = 2 * B
    in_maps = []
    for core in range(n_cores):
        b, hg = core // 2, core % 2
        hsl = slice(hg * HW, (hg + 1) * HW)
        m = {
            "xq": np.ascontiguousarray(q[b].T).astype(BF16),
            "xk": np.ascontiguousarray(k[b].T).astype(BF16),
            "xv": np.ascontiguousarray(v[b].T).astype(BF16),
            "wq": np.ascontiguousarray((w_q[hsl] * scale).T).astype(BF16),
            "wk": np.ascontiguousarray(w_k[hsl].T).astype(BF16),
            "wv": np.ascontiguousarray(w_v[hsl].T).astype(BF16),
            "bq": np.ascontiguousarray((b_q[hsl] * scale)[None, :]).astype(BF16),
            "bk": np.ascontiguousarray(b_k[hsl][None, :]).astype(BF16),
            "wo": np.ascontiguousarray(w_o[:, hsl].T).astype(BF16),
        }
        in_maps.append(m)

    res = run_bass_kernel_spmd(nc, in_maps, list(range(n_cores)), trace=TRACE)

    bias = b_o + w_o @ b_v
    out = np.empty((B, S, D), F32)
    for b in range(B):
        yt = (res.results[2 * b]["yT"].astype(F32)
              + res.results[2 * b + 1]["yT"].astype(F32))
        out[b] = yt.T + bias
    if TRACE:
        kernel.last_exec_time_ns = res.exec_time_ns
    return out


# revision 22
# speedup vs baseline: 1.1417x; 1.1349x over previous
"""Multi-head attention (B=4, S=2048, D=1024, H=16) on 8 TRN2 NeuronCores.

Sharding: core c handles batch b = c//2 and head-group hg = c%2 (8 heads).
Tensor-parallel within the core pair of a batch: w_q/w_k/w_v column-split,
w_o row-split; host sums the two partial out-projections per batch.

Device algorithm (per core), feature-major ("transposed") layouts:
  QhT/KhT = W @ x^T in [dims, S] slabs; scores^T[k, q] per head-pair via
  row-tiled concurrent matmuls (K=64 at PE rows 0-63 / 64-127); exp on ACT
  (softmax has no max-subtraction: scores ~N(0,1)); attn@V as col-tiled
  concurrent matmuls (M=64 at PE cols 0-63 / 64-127) accumulating over key
  tiles in one PSUM bank; softmax denominators via DVE tile pre-sum + a
  replicated-ones matmul that lands the denominator pre-broadcast in PSUM;
  normalize on DVE (fast reciprocal + multiply) straight out of PSUM.
  Q/K biases are applied with K=1 rank-1 matmuls inside the projection
  accumulation; the V bias is folded into the host-side output bias
  (w_o @ b_v + b_o). Projection and out-projection matmuls are interleaved
  into the attention phase as PE fillers so the scalar engine (exp) stays
  the pacing engine.
"""

import numpy as np
import ml_dtypes
from collections import deque
from contextlib import ExitStack

import concourse.bass as bass
import concourse.tile as tile
from concourse import bacc, mybir
from concourse.bass_utils import run_bass_kernel_spmd

BF16 = ml_dtypes.bfloat16
F32 = np.float32

D = 1024
N_HEAD = 16
DH = 64
HPC = 8          # heads per core
HW = HPC * DH    # head-group width = 512
P = 128

TRACE = False    # set by test.py for profiling runs

_PROG = {}


def _build_program(S):
    dt = mybir.dt
    bf = dt.bfloat16
    f32 = dt.float32

    CH = 512                 # q-chunk width
    NCH = S // CH            # q-chunks (4)
    NT = S // P              # k-tiles (16)
    NI = D // P              # contraction tiles over model dim (8)
    NP = HPC // 2            # head pairs (4)
    NO = D // P              # out-proj o-tiles (8)

    nc = bacc.Bacc("TRN2", target_bir_lowering=False, debug=False)

    xq = nc.dram_tensor("xq", [D, S], bf, kind="ExternalInput")
    xk = nc.dram_tensor("xk", [D, S], bf, kind="ExternalInput")
    xv = nc.dram_tensor("xv", [D, S], bf, kind="ExternalInput")
    wq = nc.dram_tensor("wq", [D, HW], bf, kind="ExternalInput")
    wk = nc.dram_tensor("wk", [D, HW], bf, kind="ExternalInput")
    wv = nc.dram_tensor("wv", [D, HW], bf, kind="ExternalInput")
    bq = nc.dram_tensor("bq", [1, HW], bf, kind="ExternalInput")
    bk = nc.dram_tensor("bk", [1, HW], bf, kind="ExternalInput")
    wo = nc.dram_tensor("wo", [HW, D], bf, kind="ExternalInput")
    yT = nc.dram_tensor("yT", [D, S], bf, kind="ExternalOutput")

    AF = mybir.ActivationFunctionType

    with tile.TileContext(nc) as tc:
        with ExitStack() as ctx:
            consts = ctx.enter_context(tc.tile_pool(name="consts", bufs=1))
            wpool = ctx.enter_context(tc.tile_pool(name="wpool", bufs=1))
            xpool = ctx.enter_context(tc.tile_pool(name="xpool", bufs=1))
            slabs = ctx.enter_context(tc.tile_pool(name="slabs", bufs=1))
            epool = ctx.enter_context(tc.tile_pool(name="epool", bufs=6))
            dpool = ctx.enter_context(tc.tile_pool(name="dpool", bufs=3))
            rpool = ctx.enter_context(tc.tile_pool(name="rpool", bufs=2))
            ypool = ctx.enter_context(tc.tile_pool(name="ypool", bufs=2))
            ps_sc = ctx.enter_context(
                tc.tile_pool(name="ps_sc", bufs=2, space="PSUM"))
            ps_acc = ctx.enter_context(
                tc.tile_pool(name="ps_acc", bufs=2, space="PSUM"))
            ps_proj = ctx.enter_context(
                tc.tile_pool(name="ps_proj", bufs=2, space="PSUM"))

            # ---- constants ----
            ones_row = consts.tile([1, CH], bf)
            nc.vector.memset(ones_row[:], 1.0)
            ones64 = consts.tile([P, DH], bf)
            nc.vector.memset(ones64[:], 1.0)
            warm = consts.tile([1, 16], f32)
            nc.vector.memset(warm[:], 0.0)
            bq_sb = consts.tile([1, HW], bf)
            bk_sb = consts.tile([1, HW], bf)

            # ---- weights / activations (SBUF) ----
            wq_sb = wpool.tile([P, NI, HW], bf)
            wk_sb = wpool.tile([P, NI, HW], bf)
            wv_sb = wpool.tile([P, NI, HW], bf)
            wo_sb = wpool.tile([P, NP, D], bf)
            xk_sb = xpool.tile([P, NI, S], bf)
            xq_sb = xpool.tile([P, NI, S], bf)
            xv_sb = xpool.tile([P, NI, 1024], bf)   # half-resident, 2 waves

            q_slab = slabs.tile([P, NP, S], bf)
            k_slab = slabs.tile([P, NP, S], bf)
            v_sb = slabs.tile([P, NT, HW], bf)
            attn_sb = slabs.tile([P, NP, S], bf)

            # ---- DMA schedule ----
            # ALL transfers ride the sync queue: that is HWDGE. A gpsimd
            # dma_start is SWDGE (Q7-generated descriptors) and measured
            # ~73us for 6MB -- it starved iteration 0 of wv/xv.
            # Order = consumption order of iteration 0.
            nc.sync.dma_start(bk_sb[:], bk.ap())
            nc.sync.dma_start(bq_sb[:], bq.ap())
            for i in range(NI):
                nc.sync.dma_start(wk_sb[:, i, :], wk.ap()[i * P:(i + 1) * P, :])
            for i in range(NI):
                nc.sync.dma_start(xk_sb[:, i, :], xk.ap()[i * P:(i + 1) * P, :])
            for i in range(NI):
                nc.sync.dma_start(wq_sb[:, i, :], wq.ap()[i * P:(i + 1) * P, :])
            for i in range(NI):
                nc.sync.dma_start(xq_sb[:, i, :], xq.ap()[i * P:(i + 1) * P, :])
            for i in range(NI):
                nc.sync.dma_start(wv_sb[:, i, :], wv.ap()[i * P:(i + 1) * P, :])
            for i in range(NI):   # xv wave 0 (tokens 0-1023)
                nc.sync.dma_start(xv_sb[:, i, :],
                                  xv.ap()[i * P:(i + 1) * P, 0:1024])
            for c in range(NP):
                nc.sync.dma_start(wo_sb[:, c, :], wo.ap()[c * P:(c + 1) * P, :])

            def xv_wave1():
                for i in range(NI):   # overwrites wave 0 (gated on V(t7))
                    nc.sync.dma_start(xv_sb[:, i, :],
                                      xv.ap()[i * P:(i + 1) * P, 1024:2048])

            # preload the exp table set early
            nc.scalar.activation(warm[:], warm[:], AF.Exp)

            # ---- filler tasks ----
            def k_task(p, tcc):
                tsl = slice(tcc * CH, (tcc + 1) * CH)

                def go():
                    ps = ps_proj.tile([P, CH], f32, tag="proj")
                    for i in range(NI):
                        nc.tensor.matmul(
                            ps[:], lhsT=wk_sb[:, i, p * P:(p + 1) * P],
                            rhs=xk_sb[:, i, tsl], start=(i == 0), stop=False)
                    nc.tensor.matmul(
                        ps[:], lhsT=bk_sb[0:1, p * P:(p + 1) * P],
                        rhs=ones_row[:], start=False, stop=True)
                    nc.vector.tensor_copy(k_slab[:, p, tsl], ps[:])
                return go

            def q_task(p, c):
                csl = slice(c * CH, (c + 1) * CH)

                def go():
                    ps = ps_proj.tile([P, CH], f32, tag="proj")
                    for i in range(NI):
                        nc.tensor.matmul(
                            ps[:], lhsT=wq_sb[:, i, p * P:(p + 1) * P],
                            rhs=xq_sb[:, i, csl], start=(i == 0), stop=False)
                    nc.tensor.matmul(
                        ps[:], lhsT=bq_sb[0:1, p * P:(p + 1) * P],
                        rhs=ones_row[:], start=False, stop=True)
                    nc.vector.tensor_copy(q_slab[:, p, csl], ps[:])
                return go

            def v_task(t):
                tsl = slice((t % 8) * P, (t % 8 + 1) * P)

                def go():
                    ps = ps_proj.tile([P, HW], f32, tag="proj")
                    for i in range(NI):
                        nc.tensor.matmul(
                            ps[:], lhsT=xv_sb[:, i, tsl], rhs=wv_sb[:, i, :],
                            start=(i == 0), stop=(i == NI - 1))
                    nc.vector.tensor_copy(v_sb[:, t, :], ps[:])
                return go

            def o_task(o, c):
                csl = slice(c * CH, (c + 1) * CH)

                def go():
                    ps = ps_proj.tile([P, CH], f32, tag="proj")
                    for ci in range(NP):
                        nc.tensor.matmul(
                            ps[:], lhsT=wo_sb[:, ci, o * P:(o + 1) * P],
                            rhs=attn_sb[:, ci, csl],
                            start=(ci == 0), stop=(ci == NP - 1))
                    st = ypool.tile([P, CH], bf, tag="yst")
                    nc.vector.tensor_copy(st[:], ps[:])
                    nc.sync.dma_start(yT.ap()[o * P:(o + 1) * P, csl], st[:])
                return go

            fillers = deque()

            def pump(n=1):
                for _ in range(n):
                    if fillers:
                        fillers.popleft()()

            # ---- attention iteration ----
            # The softmax epilogue (denominator matmuls -> reciprocal ->
            # normalize) of iteration N is deferred into iteration N+1's
            # kt==2 slot: its denominator matmuls wait on the DVE pre-sum
            # chain, and emitting them at the iteration boundary would
            # head-of-line-block the PE queue for ~2.8us per iteration.
            pending_epi = [None]

            def attention(p, c, kt_hook=None, pump_every=3):
                csl = slice(c * CH, (c + 1) * CH)
                hA, hB = 2 * p, 2 * p + 1
                acc = ps_acc.tile([P, CH], f32, tag="acc")
                dprev = None
                pend = deque()

                def issue_av(et, kt):
                    nc.tensor.matmul(
                        acc[0:64, :], lhsT=v_sb[:, kt, hA * DH:(hA + 1) * DH],
                        rhs=et[:, 0:CH], start=(kt == 0), stop=(kt == NT - 1),
                        tile_position=(0, 0))
                    nc.tensor.matmul(
                        acc[64:128, :], lhsT=v_sb[:, kt, hB * DH:(hB + 1) * DH],
                        rhs=et[:, CH:2 * CH], start=(kt == 0),
                        stop=(kt == NT - 1), tile_position=(0, 64))

                for kt in range(NT):
                    if kt == 2 and pending_epi[0] is not None:
                        pending_epi[0]()
                        pending_epi[0] = None
                    if kt_hook is not None:
                        kt_hook(kt)
                    ksl = slice(kt * P, (kt + 1) * P)
                    ps = ps_sc.tile([P, 2 * CH], f32, tag="sc")
                    nc.tensor.matmul(
                        ps[:, 0:CH], lhsT=k_slab[0:64, p, ksl],
                        rhs=q_slab[0:64, p, csl],
                        start=True, stop=True, tile_position=(0, 0))
                    nc.tensor.matmul(
                        ps[:, CH:2 * CH], lhsT=k_slab[64:128, p, ksl],
                        rhs=q_slab[64:128, p, csl],
                        start=True, stop=True, tile_position=(64, 0))
                    et = epool.tile([P, 2 * CH], bf, tag="et")
                    nc.scalar.activation(et[:], ps[:], AF.Exp)
                    pend.append((et, kt))
                    if len(pend) == 3:
                        issue_av(*pend.popleft())
                    dcur = dpool.tile([P, 2 * CH], bf, tag="dsum")
                    if kt == 0:
                        nc.vector.tensor_copy(dcur[:], et[:])
                    else:
                        nc.vector.tensor_add(dcur[:], dprev[:], et[:])
                    dprev = dcur
                    if kt_hook is None and kt % pump_every == 0:
                        pump()
                while pend:
                    issue_av(*pend.popleft())

                dsum = dprev

                def epilogue():
                    # denominators, pre-broadcast: rows 0-63 <- A, 64-127 <- B
                    dps = ps_sc.tile([P, 2 * CH], f32, tag="sc")
                    nc.tensor.matmul(dps[0:64, 0:CH], lhsT=ones64[:],
                                     rhs=dsum[:, 0:CH], start=True, stop=True,
                                     tile_position=(0, 0))
                    nc.tensor.matmul(dps[64:128, 0:CH], lhsT=ones64[:],
                                     rhs=dsum[:, CH:2 * CH], start=True,
                                     stop=True, tile_position=(0, 64))
                    rcp = rpool.tile([P, CH], f32, tag="rcp")
                    nc.vector.reciprocal_approx_fast(rcp[:], dps[:, 0:CH])
                    nc.vector.tensor_mul(attn_sb[:, p, csl], acc[:], rcp[:])

                pending_epi[0] = epilogue

            # ---- prefix: first V tiles, K(p0) chunk 0, Q(p0,c0) ----
            v_task(0)()
            v_task(1)()
            v_task(2)()
            k_task(0, 0)()
            q_task(0, 0)()

            # ---- iteration 0: (p0, c0) with V/K just-in-time ----
            k_remaining0 = deque([k_task(0, tcc) for tcc in (1, 2, 3)])

            def hook0(kt):
                # V(t) three steps ahead of its AV; K(p0) chunks ahead of use
                if kt < NT - 3:
                    v_task(kt + 3)()
                    if kt + 3 == 7:
                        xv_wave1()
                if kt in (2, 6, 10) and k_remaining0:
                    k_remaining0.popleft()()
                if kt % 3 == 0:
                    pump()

            # Q(0,1) must drain inside iteration (0,0): with p-outer order
            # (p,c+1) directly follows (p,c), so each iteration pumps the
            # NEXT chunk's Q projection of its own pair.
            fillers.append(q_task(0, 1))
            attention(0, 0, kt_hook=hook0)

            # remaining iterations, p-outer / c-inner: a new pair's K
            # projection is only due 4 iterations after it is queued, so
            # early iterations stay exp-paced instead of projection-bound.
            for p in range(NP):
                for c in range(NCH):
                    if p == 0 and c == 0:
                        continue  # ran above with the V/K hook
                    if c + 1 < NCH:
                        fillers.append(q_task(p, c + 1))
                    if c == 1 and p + 1 < NP:
                        for tcc in range(NCH):
                            fillers.append(k_task(p + 1, tcc))
                        fillers.append(q_task(p + 1, 0))
                    if p == NP - 1 and c > 0:
                        for o in range(NO):
                            fillers.append(o_task(o, c - 1))
                    attention(p, c, pump_every=(2 if p == NP - 1 else 3))

            # tail: last epilogue, remaining fillers, final out-proj
            if pending_epi[0] is not None:
                pending_epi[0]()
                pending_epi[0] = None
            while fillers:
                pump()
            for o in range(NO):
                o_task(o, NCH - 1)()

    nc.compile()
    return nc


def _get_program(S):
    if S not in _PROG:
        _PROG[S] = _build_program(S)
    return _PROG[S]


def enable_trace():
    """Register the NTFF profiling hook (axon images lack antenv.axon_hooks)
    and neuter the cloud artifact upload; then TRACE=True runs return
    exec_time_ns."""
    global TRACE
    import sys
    import types
    import antenv
    if "antenv.axon_hooks" not in sys.modules:
        _m = types.ModuleType("antenv.axon_hooks")
        _m._hook = None
        _m.set_axon_ntff_profile_hook = lambda h: setattr(_m, "_hook", h)
        _m.get_axon_ntff_profile_hook = lambda: _m._hook
        sys.modules["antenv.axon_hooks"] = _m
        antenv.axon_hooks = _m
        from trn_agent_boot.trn_boot import _ntff_profile_via_ctypes
        _m._hook = _ntff_profile_via_ctypes("/opt/axon/libaxon_pjrt.so")
    import concourse.bass_utils as bu
    bu.upload_artifacts = lambda tmpdir: tmpdir
    TRACE = True


def kernel(q, k, v, w_q, b_q, w_k, b_k, w_v, b_v, w_o, b_o):
    q, k, v = (np.asarray(a, F32) for a in (q, k, v))
    w_q, b_q, w_k, b_k = (np.asarray(a, F32) for a in (w_q, b_q, w_k, b_k))
    w_v, b_v, w_o, b_o = (np.asarray(a, F32) for a in (w_v, b_v, w_o, b_o))
    B, S, _ = q.shape

    nc = _get_program(S)

    scale = 1.0 / np.sqrt(np.float32(DH))
    n_cores = 2 * B
    in_maps = []
    for core in range(n_cores):
        b, hg = core // 2, core % 2
        hsl = slice(hg * HW, (hg + 1) * HW)
        m = {
            "xq": np.ascontiguousarray(q[b].T).astype(BF16),
            "xk": np.ascontiguousarray(k[b].T).astype(BF16),
            "xv": np.ascontiguousarray(v[b].T).astype(BF16),
            "wq": np.ascontiguousarray((w_q[hsl] * scale).T).astype(BF16),
            "wk": np.ascontiguousarray(w_k[hsl].T).astype(BF16),
            "wv": np.ascontiguousarray(w_v[hsl].T).astype(BF16),
            "bq": np.ascontiguousarray((b_q[hsl] * scale)[None, :]).astype(BF16),
            "bk": np.ascontiguousarray(b_k[hsl][None, :]).astype(BF16),
            "wo": np.ascontiguousarray(w_o[:, hsl].T).astype(BF16),
        }
        in_maps.append(m)

    res = run_bass_kernel_spmd(nc, in_maps, list(range(n_cores)), trace=TRACE)

    bias = b_o + w_o @ b_v
    out = np.empty((B, S, D), F32)
    for b in range(B):
        yt = (res.results[2 * b]["yT"].astype(F32)
              + res.results[2 * b + 1]["yT"].astype(F32))
        out[b] = yt.T + bias
    if TRACE:
        kernel.last_exec_time_ns = res.exec_time_ns
    return out
